# revision 1
# baseline (speedup 1.0000x reference)
"""NexusNet GNN message-passing kernel for 8 Trainium2 NeuronCores.

Sharding:
  - nexus_up + nexus MLP: sharded by nexus node (M/8 contiguous segs/core);
    edges routed to the core owning their dst segment (host index prep).
    Aggregation via one-hot matmul on PE into PSUM per 128-seg block.
  - n [M,C,FN] (+ per-plane edge-logit b terms) AllGathered to every core.
  - nexus_down: sharded by planar node (N/8 per core, 2 halves/core/plane).
    Per-edge msg = softmax(logit) * n[dst]; logit = a[src] + b[dst] where
    a is a dense per-node dot(x, We) table.  Scatter-mean by src done with
    dma_scatter_add over CSR-slot-ordered edges (unique idx per call).
  - Final 2-layer MLP feature-major on PE; output transposed on host.
"""

import numpy as np

import concourse.bass as bass
import concourse.bacc as bacc
import concourse.mybir as mybir
import concourse.tile as tile

F32 = mybir.dt.float32
F32R = mybir.dt.float32r
I32 = mybir.dt.int32
I16 = mybir.dt.int16
TANH = mybir.ActivationFunctionType.Tanh
EXP = mybir.ActivationFunctionType.Exp
ALU = mybir.AluOpType

CFG_FULL = dict(P=3, N=100000, M=30000, E=200000, C=5, FP=64, FN=32, NC=8)

B_SC = 1024           # edges per down-phase gather/scatter call
NROW = 192            # padded n-row floats (160 n + 15 b + 17 pad)
AROW = 64             # padded a-row floats (5 a + 1 invdeg + pad)
GRP = 4               # up-phase seg blocks per nexus-MLP group
CHW = 512             # stage-C m chunk width


def _ceil(a, b):
    return (a + b - 1) // b


def _wrap16(a):
    # flat idx j -> (partition j%16, col j//16), replicated to 128 partitions
    w = a.reshape(-1, 16).T.copy()
    return np.tile(w, (8, 1))


def host_prep(inputs, cfg):
    P, N, M, E, C, FP, FN, NC = (cfg[k] for k in
                                 ("P", "N", "M", "E", "C", "FP", "FN", "NC"))
    M_LOC = M // NC
    N_LOC = N // NC
    NH = N_LOC // 2                       # nodes per half
    NHP = _ceil(NH, 128) * 128            # padded half (6272)
    NB = _ceil(M_LOC, 128)                # up seg blocks per core
    NTAB = NHP + 128                      # table rows (+trash region)
    TRASH = NTAB - 1

    x = np.ascontiguousarray(np.asarray(inputs["x"], np.float32)
                             .reshape(P, N, C * FP))
    esrc = np.asarray(inputs["edge_src"])
    edst = np.asarray(inputs["edge_dst"])

    # per-core feature-major x slices: [P, 2, C*FP, NH]
    xloc = x.reshape(P, NC, 2, NH, C * FP).transpose(1, 0, 2, 4, 3)
    xloc = np.ascontiguousarray(xloc, np.float32)
    x_flat = x.reshape(P * N, C * FP)

    # ---------------- UP phase indices ----------------
    per_kp = {}
    max_blk_cnt = 0
    for p in range(P):
        order = np.argsort(edst[p], kind="stable")
        ds, ss = edst[p][order], esrc[p][order]
        bounds = np.searchsorted(ds, np.arange(NC + 1) * M_LOC)
        for k in range(NC):
            sl = slice(bounds[k], bounds[k + 1])
            dsl = (ds[sl] - k * M_LOC).astype(np.int64)
            blk = dsl >> 7
            cnt = np.bincount(blk, minlength=NB)
            max_blk_cnt = max(max_blk_cnt, int(cnt.max(initial=0)))
            per_kp[(k, p)] = (dsl, (ss[sl] + p * N).astype(np.int64), blk, cnt)
    K_UP = max(1, _ceil(max_blk_cnt, 128))
    NBK = NB * K_UP

    up_src = np.zeros((NC, P, NBK * 128), np.int32)
    up_dr = np.full((NC, P, NBK * 128), -1.0, np.float32)
    for (k, p), (dsl, sglob, blk, cnt) in per_kp.items():
        starts = np.concatenate(([0], np.cumsum(cnt)))[:-1]
        r = np.arange(len(dsl)) - np.repeat(starts, cnt)
        pos = blk * (K_UP * 128) + r
        up_src[k, p, pos] = sglob
        up_dr[k, p, pos] = dsl - (blk << 7)
    up_src = up_src.reshape(NC, P, NBK, 128).transpose(0, 1, 3, 2).copy()
    up_dr = up_dr.reshape(NC, P, NBK, 128).transpose(0, 1, 3, 2).copy()

    # ---------------- DOWN phase indices ----------------
    down = {}
    slot_cnt_all = []
    for p in range(P):
        order = np.argsort(esrc[p], kind="stable")
        ss, dd = esrc[p][order], edst[p][order]
        bounds = np.searchsorted(ss, np.arange(2 * NC + 1) * NH)
        for j in range(2 * NC):
            k, h = j // 2, j % 2
            sl = slice(bounds[j], bounds[j + 1])
            s_loc = (ss[sl] - j * NH).astype(np.int64)
            d_loc = dd[sl].astype(np.int64)
            deg = np.bincount(s_loc, minlength=NH)
            starts = np.concatenate(([0], np.cumsum(deg)))[:-1]
            rank = np.arange(len(s_loc)) - np.repeat(starts, deg)
            o2 = np.lexsort((s_loc, rank))
            s2, d2, r2 = s_loc[o2], d_loc[o2], rank[o2]
            scnt = (np.bincount(r2) if len(r2) else np.zeros(1, np.int64))
            slot_cnt_all.append(scnt)
            down[(k, p, h)] = (s2, d2, scnt, deg)
    S_MAX = max(len(s) for s in slot_cnt_all)
    gmax = np.zeros(S_MAX, np.int64)
    for s in slot_cnt_all:
        gmax[: len(s)] = np.maximum(gmax[: len(s)], s)
    calls_s = np.array([_ceil(int(g), B_SC) for g in gmax if g > 0])
    call_off = np.concatenate(([0], np.cumsum(calls_s)))
    NCALLS = int(call_off[-1])
    L = NCALLS * B_SC

    dn_dst = np.zeros((NC, 2 * P, 128, L // 16), np.int16)
    dn_srel = np.zeros((NC, 2 * P, 128, L // 16), np.int16)
    dn_scat = np.zeros((NC, 2 * P, 128, L // 16), np.int16)
    degf = np.ones((NC, 2 * P, NTAB), np.float32)
    for (k, p, h), (s2, d2, scnt, deg) in down.items():
        ph = p * 2 + h
        dstA = np.zeros(L, np.int16)
        srelA = np.zeros(L, np.int16)
        scatA = np.full(L, TRASH, np.int16)
        sstart = np.concatenate(([0], np.cumsum(scnt)))[:-1]
        j = np.arange(len(s2)) - np.repeat(sstart, scnt)
        pos = np.repeat(call_off[: len(scnt)] * B_SC, scnt) + j
        dstA[pos] = d2
        srelA[pos] = s2
        scatA[pos] = s2
        dn_dst[k, ph] = _wrap16(dstA)
        dn_srel[k, ph] = _wrap16(srelA)
        dn_scat[k, ph] = _wrap16(scatA)
        degf[k, ph, :NH] = np.maximum(deg, 1).astype(np.float32)
    # deg layout: [128, 2P*(NTAB//128)]: (r, ph*(NTAB//128)+t) = deg[ph][t*128+r]
    degw = (degf.reshape(NC, 2 * P, NTAB // 128, 128)
            .transpose(0, 3, 1, 2).reshape(NC, 128, -1).copy())

    # ---------------- weights ----------------
    g = lambda n: np.asarray(inputs[n], np.float32)
    Wn1, Wn2, We, Wd1, Wd2 = g("Wn1"), g("Wn2"), g("We"), g("Wd1"), g("Wd2")
    bn1, bn2, be, bd1, bd2 = g("bn1"), g("bn2"), g("be"), g("bd1"), g("bd2")

    wn1t = np.stack([Wn1.transpose(2, 0, 1)[p * FP:(p + 1) * FP]
                     .reshape(FP, C * FN) for p in range(P)]).copy()
    wn2t = Wn2.transpose(2, 0, 1).reshape(FN, C * FN).copy()
    # b-term weights: block-diagonal for classes 0..3 (K = 4*FN) and an
    # augmented [FN+1] block for class 4 whose ones-row adds be for all cols.
    went = We[:, :, 0, FP:]                                   # [P, C, FN]
    wentA = np.zeros((4 * FN, C * P), np.float32)
    for c in range(4):
        wentA[c * FN:(c + 1) * FN, c * P:(c + 1) * P] = went[:, c, :].T
    wentB = np.zeros((FN + 1, C * P), np.float32)
    wentB[:FN, 4 * P:] = went[:, 4, :].T
    wentB[FN, :] = be[:, :, 0].T.reshape(-1)
    bn1c = bn1.reshape(C, FN, 1).copy()
    bn2c = bn2.reshape(C, FN, 1).copy()
    we1 = We[:, :, 0, :FP].transpose(0, 2, 1).copy()          # [P, FP, C]
    wd1t = Wd1.transpose(0, 3, 1, 2).reshape(P, FP + FN, C * FP).copy()
    wd2t = Wd2.transpose(0, 1, 3, 2).copy()                   # [P, C, FP, FP]
    bd1c = bd1.reshape(P, C, FP, 1).copy()
    bd2c = bd2.reshape(P, C, FP, 1).copy()
    iota = np.tile(np.arange(128, dtype=np.float32), (128, 1)).copy()
    ident = np.eye(128, dtype=np.float32)

    meta = dict(cfg=cfg, M_LOC=M_LOC, N_LOC=N_LOC, NH=NH, NHP=NHP,
                NB=NB, K_UP=K_UP, NBK=NBK, NTAB=NTAB, TRASH=TRASH,
                NCALLS=NCALLS, L=L, S_MAX=S_MAX)

    shared = dict(x=x_flat, wn1t=wn1t, wn2t=wn2t, wentA=wentA, wentB=wentB,
                  bn1c=bn1c, bn2c=bn2c, we1=we1, wd1t=wd1t, wd2t=wd2t,
                  bd1c=bd1c, bd2c=bd2c, iota=iota, ident=ident)
    in_maps = []
    for k in range(NC):
        m = dict(shared)
        m.update(xloc=xloc[k], up_src=up_src[k], up_dr=up_dr[k],
                 dn_dst=dn_dst[k], dn_srel=dn_srel[k], dn_scat=dn_scat[k],
                 degw=degw[k])
        in_maps.append(m)
    return in_maps, meta


def build_kernel(meta):
    cfg = meta["cfg"]
    P, N, M, E, C, FP, FN, NC = (cfg[k] for k in
                                 ("P", "N", "M", "E", "C", "FP", "FN", "NC"))
    M_LOC, NH, NHP = meta["M_LOC"], meta["NH"], meta["NHP"]
    NMT = NHP // 128
    NB, K_UP, NBK = meta["NB"], meta["K_UP"], meta["NBK"]
    NTAB, NCALLS, L = meta["NTAB"], meta["NCALLS"], meta["L"]
    CF = C * FP
    CN = C * FN
    NBW = FN + C * P           # nbt rows: class-4 n (FN) + b stack (C*P)
    assert C == 5

    nc = bacc.Bacc("TRN2", num_devices=NC)

    def param(name, shape, dt=F32, out=False):
        return nc.declare_dram_parameter(name, list(shape), dt, isOutput=out)

    x_d = param("x", [P * N, CF])
    xloc_d = param("xloc", [P, 2, CF, NH])
    up_src_d = param("up_src", [P, 128, NBK], I32)
    up_dr_d = param("up_dr", [P, 128, NBK])
    dn_dst_d = param("dn_dst", [2 * P, 128, L // 16], I16)
    dn_srel_d = param("dn_srel", [2 * P, 128, L // 16], I16)
    dn_scat_d = param("dn_scat", [2 * P, 128, L // 16], I16)
    degw_d = param("degw", [128, 2 * P * (NTAB // 128)])
    wn1t_d = param("wn1t", [P, FP, CN])
    wn2t_d = param("wn2t", [FN, CN])
    wentA_d = param("wentA", [4 * FN, C * P])
    wentB_d = param("wentB", [FN + 1, C * P])
    bn1c_d = param("bn1c", [C, FN, 1])
    bn2c_d = param("bn2c", [C, FN, 1])
    we1_d = param("we1", [P, FP, C])
    wd1t_d = param("wd1t", [P, FP + FN, C * FP])
    wd2t_d = param("wd2t", [P, C, FP, FP])
    bd1c_d = param("bd1c", [P, C, FP, 1])
    bd2c_d = param("bd2c", [P, C, FP, 1])
    iota_d = param("iota", [128, 128])
    ident_d = param("ident", [128, 128])
    out_d = param("outT", [P, 2, C, FP, NHP], out=True)

    n_loc = nc.dram_tensor("n_loc", [M_LOC, NROW], F32)
    n_full = nc.dram_tensor("n_full", [NC * M_LOC, NROW], F32,
                            addr_space="Shared")
    a_tabs = [nc.dram_tensor(f"a_tab{i}", [NTAB, AROW], F32)
              for i in range(2 * P)]
    s_tabs = [nc.dram_tensor(f"s_tab{i}", [NTAB, NROW], F32)
              for i in range(2 * P)]

    with tile.TileContext(nc) as tc:
        with tc.tile_pool(name="const", bufs=1) as cp:
            iota_t = cp.tile([128, 128], F32R)
            nc.sync.dma_start(out=iota_t[:], in_=iota_d[:].bitcast(F32R))
            ident_t = cp.tile([128, 128], F32)
            nc.sync.dma_start(out=ident_t[:], in_=ident_d[:])
            wn1t_t = [cp.tile([FP, CN], F32R, name=f"wn1t{p}")
                      for p in range(P)]
            wn2t_t = cp.tile([FN, CN], F32R)
            wentA_t = cp.tile([4 * FN, C * P], F32R)
            wentB_t = cp.tile([FN + 1, C * P], F32R)
            nc.sync.dma_start(out=wn2t_t[:], in_=wn2t_d[:].bitcast(F32R))
            nc.sync.dma_start(out=wentA_t[:], in_=wentA_d[:].bitcast(F32R))
            nc.sync.dma_start(out=wentB_t[:], in_=wentB_d[:].bitcast(F32R))
            bn1c_t = [cp.tile([FN, 1], F32, name=f"bn1c{c}") for c in range(C)]
            bn2c_t = [cp.tile([FN, 1], F32, name=f"bn2c{c}") for c in range(C)]
            we1_t = [cp.tile([FP, C], F32, name=f"we1{p}") for p in range(P)]
            wd1t_t = [cp.tile([FP + FN, C * FP], F32R, name=f"wd1t{p}")
                      for p in range(P)]
            wd2t_t = [[cp.tile([FP, FP], F32R, name=f"wd2t{p}_{c}")
                       for c in range(C)] for p in range(P)]
            bd1c_t = [[cp.tile([FP, 1], F32, name=f"bd1c{p}_{c}")
                       for c in range(C)] for p in range(P)]
            bd2c_t = [[cp.tile([FP, 1], F32, name=f"bd2c{p}_{c}")
                       for c in range(C)] for p in range(P)]
            for p in range(P):
                nc.sync.dma_start(out=wn1t_t[p][:], in_=wn1t_d[p].bitcast(F32R))
                nc.sync.dma_start(out=we1_t[p][:], in_=we1_d[p])
                nc.sync.dma_start(out=wd1t_t[p][:], in_=wd1t_d[p].bitcast(F32R))
                for c in range(C):
                    nc.sync.dma_start(out=wd2t_t[p][c][:],
                                      in_=wd2t_d[p, c].bitcast(F32R))
                    nc.sync.dma_start(out=bd1c_t[p][c][:], in_=bd1c_d[p, c])
                    nc.sync.dma_start(out=bd2c_t[p][c][:], in_=bd2c_d[p, c])
            for c in range(C):
                nc.sync.dma_start(out=bn1c_t[c][:], in_=bn1c_d[c])
                nc.sync.dma_start(out=bn2c_t[c][:], in_=bn2c_d[c])
            upsrc_t = [cp.tile([128, NBK], I32, name=f"upsrc{p}")
                       for p in range(P)]
            updr_t = [cp.tile([128, NBK], F32, name=f"updr{p}")
                      for p in range(P)]
            for p in range(P):
                nc.scalar.dma_start(out=upsrc_t[p][:], in_=up_src_d[p])
                nc.scalar.dma_start(out=updr_t[p][:], in_=up_dr_d[p])
            degw_t = cp.tile([128, 2 * P * (NTAB // 128)], F32)
            nc.scalar.dma_start(out=degw_t[:], in_=degw_d[:])

            # zero-init s tables
            zt = cp.tile([128, NROW], F32)
            nc.vector.memset(zt[:], 0.0)
            ones_f = cp.tile([1, GRP * 128], F32)
            nc.vector.memset(ones_f[:], 1.0)
            ones_r = cp.tile([1, GRP * 128], F32R)
            nc.vector.tensor_copy(out=ones_r[:], in_=ones_f[:])
            zeros_r = cp.tile([128, 64], F32R)
            nc.vector.tensor_copy(out=zeros_r[:], in_=zt[:, :64])
            for i in range(2 * P):
                st3 = s_tabs[i].ap().rearrange("(t q) r -> t q r", q=128)
                for t in range(NTAB // 128):
                    nc.sync.dma_start(out=st3[t], in_=zt[:])

            # ======================= UP PHASE =======================
            n_loc_ap = n_loc.ap()
            with tc.tile_pool(name="up_sb", bufs=3) as up, \
                 tc.tile_pool(name="up_sb1", bufs=2) as up1, \
                 tc.tile_pool(name="up_ps", bufs=2, space="PSUM") as upp, \
                 tc.tile_pool(name="up_ps1", bufs=1, space="PSUM") as upp1, \
                 tc.tile_pool(name="mlp_ps", bufs=1, space="PSUM") as mpp:
                for g0 in range(0, NB, GRP):
                    gb = list(range(g0, min(g0 + GRP, NB)))
                    GW = len(gb) * 128
                    # per-plane per-class feature-major up tiles [64, GRP*128]
                    upX = [[up1.tile([FP, GRP * 128], F32R,
                                     name=f"upX{p}_{c}", tag=f"upX{p}_{c}")
                            for c in range(C)] for p in range(P)]
                    for p in range(P):
                        for bi, b in enumerate(gb):
                            pu = upp.tile([128, CF], F32, tag="pu",
                                          space="PSUM")
                            for kk in range(K_UP):
                                col = b * K_UP + kk
                                G = up.tile([128, CF], F32R, tag="G")
                                nc.gpsimd.indirect_dma_start(
                                    out=G[:], out_offset=None,
                                    in_=x_d[:].bitcast(F32R),
                                    in_offset=bass.IndirectOffsetOnAxis(
                                        ap=upsrc_t[p][:, col:col + 1], axis=0))
                                O = up.tile([128, 128], F32R, tag="O")
                                nc.vector.tensor_tensor(
                                    out=O[:],
                                    in0=updr_t[p][:, col:col + 1]
                                        .bitcast(F32R).to_broadcast([128, 128]),
                                    in1=iota_t[:],
                                    op=ALU.is_equal)
                                nc.tensor.matmul(out=pu[:], lhsT=O[:],
                                                 rhs=G[:], start=(kk == 0),
                                                 stop=(kk == K_UP - 1))
                            stg = up.tile([128, CF], F32, tag="stg")
                            nc.scalar.copy(out=stg[:], in_=pu[:])
                            csl = slice(bi * 128, (bi + 1) * 128)
                            for ti in range(3):
                                w = min(128, CF - ti * 128)
                                pt = upp1.tile([128, 128], F32, tag="ptr",
                                               space="PSUM")
                                nc.tensor.transpose(
                                    out=pt[:w, :],
                                    in_=stg[:, ti * 128:ti * 128 + w],
                                    identity=ident_t[:])
                                nc.vector.tensor_copy(
                                    out=upX[p][2 * ti][:, csl],
                                    in_=pt[0:FP, :])
                                if 2 * ti + 1 < C:
                                    nc.vector.tensor_copy(
                                        out=upX[p][2 * ti + 1][:, csl],
                                        in_=pt[FP:2 * FP, :])
                    # ---- nexus MLP over this group ----
                    n1c = [up.tile([FN, GRP * 128], F32R, name=f"n1c{c}",
                                   tag=f"n1c{c}") for c in range(C)]
                    for c in range(C):
                        pn1 = mpp.tile([FN, GRP * 128], F32, tag="pn1",
                                       space="PSUM", bufs=2)
                        for p in range(P):
                            nc.tensor.matmul(
                                out=pn1[:, :GW],
                                lhsT=wn1t_t[p][:, c * FN:(c + 1) * FN],
                                rhs=upX[p][c][:, :GW],
                                start=(p == 0), stop=(p == P - 1))
                        nc.scalar.activation(n1c[c][:, :GW], pn1[:, :GW],
                                             TANH, bias=bn1c_t[c][:])
                    n2s = up.tile([4 * FN, GRP * 128], F32R, tag="n2s")
                    nbt = up.tile([FN + 1, GRP * 128], F32R, tag="nbt")
                    nc.vector.tensor_copy(out=nbt[FN:FN + 1, :],
                                          in_=ones_r[:])
                    for c in range(C):
                        pn2 = mpp.tile([FN, GRP * 128], F32, tag="pn2",
                                       space="PSUM", bufs=2)
                        nc.tensor.matmul(
                            out=pn2[:, :GW],
                            lhsT=wn2t_t[:, c * FN:(c + 1) * FN],
                            rhs=n1c[c][:, :GW], start=True, stop=True)
                        dst = (n2s[c * FN:(c + 1) * FN, :GW] if c < 4
                               else nbt[0:FN, :GW])
                        nc.scalar.activation(dst, pn2[:, :GW],
                                             TANH, bias=bn2c_t[c][:])
                    pbv = mpp.tile([C * P, GRP * 128], F32, tag="misc",
                                   space="PSUM", bufs=1)
                    nc.tensor.matmul(out=pbv[:, :GW], lhsT=wentA_t[:],
                                     rhs=n2s[:, :GW], start=True, stop=False)
                    nc.tensor.matmul(out=pbv[:, :GW], lhsT=wentB_t[:],
                                     rhs=nbt[:, :GW], start=False, stop=True)
                    bt = up.tile([C * P, GRP * 128], F32, tag="bt")
                    nc.vector.tensor_copy(out=bt[:, :GW], in_=pbv[:, :GW])
                    # assemble + store n rows per block
                    for bi, b in enumerate(gb):
                        rows = min(128, M_LOC - b * 128)
                        sl = slice(bi * 128, bi * 128 + 128)
                        tp = mpp.tile([128, 4 * FN + FN + C * P], F32,
                                      tag="misc", space="PSUM", bufs=1)
                        nc.tensor.transpose(
                            out=tp[:, 0:4 * FN],
                            in_=n2s[:, sl].bitcast(F32),
                            identity=ident_t[:])
                        nc.tensor.transpose(
                            out=tp[:, 4 * FN:CN],
                            in_=nbt[0:FN, sl].bitcast(F32),
                            identity=ident_t[:FN, :FN])
                        nc.tensor.transpose(
                            out=tp[:, CN:CN + C * P],
                            in_=bt[:, sl],
                            identity=ident_t[:C * P, :C * P])
                        nrow = up.tile([128, NROW], F32, tag="nrow")
                        nc.vector.tensor_copy(out=nrow[:, 0:CN + C * P],
                                              in_=tp[:])
                        nc.vector.memset(nrow[:, CN + C * P:], 0.0)
                        nc.sync.dma_start(
                            out=n_loc_ap[b * 128:b * 128 + rows, :],
                            in_=nrow[:rows, :])

            # ================= AllGather n =================
            nc.gpsimd.collective_compute(
                "AllGather", ALU.bypass,
                replica_groups=[list(range(NC))],
                ins=[n_loc.ap().opt()], outs=[n_full.ap().opt()])

            # ================= STAGE A: a tables =================
            with tc.tile_pool(name="sa_sb", bufs=2) as sa, \
                 tc.tile_pool(name="sa_ps", bufs=2, space="PSUM") as sap:
                for ph in range(2 * P):
                    p, h = ph // 2, ph % 2
                    for ch0 in range(0, NHP, CHW):
                        cw = min(CHW, NHP - ch0)
                        rw = min(max(NH - ch0, 0), cw)   # real cols
                        xtc = [sa.tile([FP, CHW], F32, name=f"xtc{c}",
                                       tag=f"xtc{c}") for c in range(C)]
                        for c in range(C):
                            if rw < cw:
                                nc.vector.memset(xtc[c][:, rw:cw], 0.0)
                            if rw > 0:
                                nc.sync.dma_start(
                                    out=xtc[c][:, :rw],
                                    in_=xloc_d[p, h, c * FP:(c + 1) * FP,
                                               ch0:ch0 + rw])
                        for j in range(cw // 128):
                            t = ch0 // 128 + j
                            pa = sap.tile([128, C], F32, tag="pa",
                                          space="PSUM")
                            for c in range(C):
                                nc.tensor.matmul(
                                    out=pa[:, c:c + 1],
                                    lhsT=xtc[c][:, j * 128:(j + 1) * 128],
                                    rhs=we1_t[p][:, c:c + 1],
                                    start=True, stop=True)
                            ast = sa.tile([128, AROW], F32, tag="ast")
                            nc.vector.memset(ast[:, C + 1:], 0.0)
                            nc.vector.tensor_copy(out=ast[:, 0:C], in_=pa[:])
                            nc.vector.reciprocal(
                                out=ast[:, C:C + 1],
                                in_=degw_t[:, ph * (NTAB // 128) + t:
                                           ph * (NTAB // 128) + t + 1])
                            nc.sync.dma_start(
                                out=a_tabs[ph].ap()[t * 128:(t + 1) * 128, :],
                                in_=ast[:])
                    for t in range(NMT, NTAB // 128):
                        nc.sync.dma_start(
                            out=a_tabs[ph].ap()[t * 128:(t + 1) * 128, :],
                            in_=zt[:, :AROW])

            # ================= STAGE B: edge stream =================
            NSL = B_SC // 128
            W16 = B_SC // 16
            with tc.tile_pool(name="sb_idx", bufs=1) as ip, \
                 tc.tile_pool(name="sb_sb", bufs=6) as sbp:
                dst_t, srel_t, scat_t = [], [], []
                for ph in range(2 * P):
                    d = ip.tile([128, L // 16], I16, name=f"dt{ph}")
                    nc.scalar.dma_start(out=d[:], in_=dn_dst_d[ph])
                    s = ip.tile([128, L // 16], I16, name=f"srt{ph}")
                    nc.scalar.dma_start(out=s[:], in_=dn_srel_d[ph])
                    sc = ip.tile([128, L // 16], I16, name=f"sct{ph}")
                    nc.scalar.dma_start(out=sc[:], in_=dn_scat_d[ph])
                    dst_t.append(d)
                    srel_t.append(s)
                    scat_t.append(sc)
                for cix in range(NCALLS):
                    for ph in range(2 * P):
                        p = ph // 2
                        isl = slice(cix * W16, (cix + 1) * W16)
                        gn = sbp.tile([128, NSL, NROW], F32, tag="gn")
                        nc.gpsimd.dma_gather(
                            out_ap=gn[:], in_ap=n_full.ap()[:],
                            idxs_ap=dst_t[ph][:, isl],
                            num_idxs=B_SC, num_idxs_reg=B_SC, elem_size=NROW)
                        ga = sbp.tile([128, NSL, AROW], F32, tag="ga")
                        nc.gpsimd.dma_gather(
                            out_ap=ga[:], in_ap=a_tabs[ph].ap()[:],
                            idxs_ap=srel_t[ph][:, isl],
                            num_idxs=B_SC, num_idxs_reg=B_SC, elem_size=AROW)
                        lg = sbp.tile([128, NSL, C], F32, tag="lg")
                        nc.vector.tensor_tensor(
                            out=lg[:], in0=ga[:, :, 0:C],
                            in1=gn[:, :, CN + p:CN + p + (C - 1) * P + 1:P],
                            op=ALU.add)
                        mx = sbp.tile([128, NSL], F32, tag="mx")
                        nc.vector.tensor_reduce(out=mx[:], in_=lg[:],
                                                axis=mybir.AxisListType.X,
                                                op=ALU.max)
                        nc.vector.tensor_tensor(
                            out=lg[:], in0=lg[:],
                            in1=mx[:].to_broadcast([128, NSL, C]),
                            op=ALU.subtract)
                        ex = sbp.tile([128, NSL, C], F32, tag="ex")
                        nc.scalar.activation(ex[:], lg[:], EXP)
                        sm = sbp.tile([128, NSL], F32, tag="sm")
                        nc.vector.tensor_reduce(out=sm[:], in_=ex[:],
                                                axis=mybir.AxisListType.X,
                                                op=ALU.add)
                        nc.vector.reciprocal(out=sm[:], in_=sm[:])
                        nc.vector.tensor_tensor(out=sm[:], in0=sm[:],
                                                in1=ga[:, :, C],
                                                op=ALU.mult)
                        nc.vector.tensor_tensor(
                            out=ex[:], in0=ex[:],
                            in1=sm[:].to_broadcast([128, NSL, C]),
                            op=ALU.mult)
                        msg = sbp.tile([128, NSL, NROW], F32, tag="msg")
                        nc.vector.memset(msg[:, :, CN:], 0.0)
                        nc.vector.tensor_tensor(
                            out=msg[:, :, 0:CN].rearrange(
                                "a b (c f) -> a b c f", f=FN),
                            in0=gn[:, :, 0:CN].rearrange(
                                "a b (c f) -> a b c f", f=FN),
                            in1=ex[:].to_broadcast([128, NSL, C, FN]),
                            op=ALU.mult)
                        nc.gpsimd.dma_scatter_add(
                            out_ap=s_tabs[ph].ap()[:], in_ap=msg[:],
                            idxs_ap=scat_t[ph][:, isl],
                            num_idxs=B_SC, num_idxs_reg=B_SC, elem_size=NROW)

            # ================= STAGE C: down MLP =================
            with tc.tile_pool(name="sc_sb", bufs=3) as scb, \
                 tc.tile_pool(name="sc_ft", bufs=1) as ftp, \
                 tc.tile_pool(name="sc_ps", bufs=2, space="PSUM") as scp:
                for ph in range(2 * P):
                    p, h = ph // 2, ph % 2
                    ft = [ftp.tile([FP + FN, NHP], F32R, name=f"ft{c}",
                                   tag=f"ft{c}") for c in range(C)]
                    for c in range(C):
                        if NHP > NH:
                            nc.vector.tensor_copy(
                                out=ft[c][:, NH:],
                                in_=zeros_r[:FP + FN, :NHP - NH])
                        nc.sync.dma_start(
                            out=ft[c][0:FP, :NH],
                            in_=xloc_d[p, h, c * FP:(c + 1) * FP, :]
                                .bitcast(F32R))
                    for t in range(NMT):
                        st = scb.tile([128, NROW], F32, tag="st")
                        nc.sync.dma_start(
                            out=st[:],
                            in_=s_tabs[ph].ap()[t * 128:(t + 1) * 128, :])
                        t1 = scp.tile([128, 128], F32, tag="st1", space="PSUM")
                        nc.tensor.transpose(out=t1[:, 0:4 * FN],
                                            in_=st[:, 0:4 * FN],
                                            identity=ident_t[:])
                        for c in range(4):
                            nc.vector.tensor_copy(
                                out=ft[c][FP:FP + FN, t * 128:(t + 1) * 128],
                                in_=t1[c * FN:(c + 1) * FN, :])
                        t2 = scp.tile([FN, 128], F32, tag="st2", space="PSUM")
                        nc.tensor.transpose(out=t2[:],
                                            in_=st[:, 4 * FN:CN],
                                            identity=ident_t[:])
                        nc.vector.tensor_copy(
                            out=ft[4][FP:FP + FN, t * 128:(t + 1) * 128],
                            in_=t2[:])
                    for ch0 in range(0, NHP, CHW):
                        cw = min(CHW, NHP - ch0)
                        csl = slice(ch0, ch0 + cw)
                        for c in range(C):
                            hps = scp.tile([FP, CHW], F32, tag="hps",
                                           space="PSUM")
                            nc.tensor.matmul(
                                out=hps[:, :cw],
                                lhsT=wd1t_t[p][:, c * FP:(c + 1) * FP],
                                rhs=ft[c][:, csl], start=True, stop=True)
                            ht = scb.tile([FP, CHW], F32R, tag="ht")
                            nc.scalar.activation(ht[:, :cw], hps[:, :cw],
                                                 TANH, bias=bd1c_t[p][c][:])
                            ops_ = scp.tile([FP, CHW], F32, tag="ops",
                                            space="PSUM")
                            nc.tensor.matmul(
                                out=ops_[:, :cw], lhsT=wd2t_t[p][c][:],
                                rhs=ht[:, :cw], start=True, stop=True)
                            ot = scb.tile([FP, CHW], F32, tag="ot")
                            nc.scalar.activation(ot[:, :cw], ops_[:, :cw],
                                                 TANH, bias=bd2c_t[p][c][:])
                            nc.sync.dma_start(
                                out=out_d[p, h, c, :, csl],
                                in_=ot[:, :cw])

    nc.compile()
    return nc


_CACHE = {}


def _get_compiled(inputs, cfg):
    in_maps, meta = host_prep(inputs, cfg)
    key = (meta["K_UP"], meta["NCALLS"], meta["S_MAX"],
           tuple(sorted(cfg.items())))
    if key not in _CACHE:
        _CACHE[key] = build_kernel(meta)
    return _CACHE[key], in_maps, meta


def assemble_output(results, meta):
    cfg = meta["cfg"]
    P, N, C, FP, NC = (cfg[k] for k in ("P", "N", "C", "FP", "NC"))
    NH = meta["NH"]
    # results[k]["outT"]: [P, 2, C, FP, NHP]
    arr = np.stack([np.asarray(results[k]["outT"])[:, :, :, :, :NH]
                    for k in range(NC)])
    # [NC, P, 2, C, FP, NH] -> [P, NC, 2, NH, C, FP]
    out = arr.transpose(1, 0, 2, 5, 3, 4).reshape(P, N, C, FP)
    return np.ascontiguousarray(out)


def kernel(**inputs):
    from concourse.bass_utils import run_bass_kernel_spmd
    cfg = CFG_FULL
    nc, in_maps, meta = _get_compiled(inputs, cfg)
    res = run_bass_kernel_spmd(nc, in_maps, list(range(cfg["NC"])))
    return assemble_output(res.results, meta)



# revision 17
# speedup vs baseline: 1.2011x; 1.2011x over previous
"""NexusNet GNN message-passing kernel for 8 Trainium2 NeuronCores.

Sharding:
  - nexus_up + nexus MLP: sharded by nexus node (M/8 contiguous segs/core);
    edges routed to the core owning their dst segment (host index prep).
    Aggregation via one-hot matmul on PE into PSUM per 128-seg block.
  - n [M,C,FN] (+ per-plane edge-logit b terms) AllGathered to every core.
  - nexus_down: sharded by planar node (N/8 per core per plane).  Edges
    src-sorted into 512-src groups, 128-edge tiles.  Per edge: gather the
    192-float n-row (160 n + 15 b) by dst via indirect DMA; build a one-hot
    [edge, src-in-group]; expand per-src a-values (+invdeg) to edges via
    4 transposed-one-hot matmuls; softmax weights; aggregate msgs with
    feature-major one-hot matmuls straight into PSUM (no scatter).
  - a-table: dense block-diag matmul over feature-major x (a = x . We).
  - Final 2-layer MLP feature-major on PE; output transposed on host.
"""

import numpy as np
import ml_dtypes

import concourse.bass as bass
import concourse.bacc as bacc
import concourse.mybir as mybir
import concourse.tile as tile

F32 = mybir.dt.float32
F32R = mybir.dt.float32r
BF16 = mybir.dt.bfloat16
I32 = mybir.dt.int32
I16 = mybir.dt.int16
TANH = mybir.ActivationFunctionType.Tanh
EXP = mybir.ActivationFunctionType.Exp
ALU = mybir.AluOpType

CFG_FULL = dict(P=3, N=100000, M=30000, E=200000, C=5, FP=64, FN=32, NC=8)

NROW = 192            # n-row floats (160 n + 15 b + 17 pad)
GRP = 4               # up-phase seg blocks per nexus-MLP group
SG = 512              # down-phase src group width
CHW = 512             # stage A/C node chunk width


def _ceil(a, b):
    return (a + b - 1) // b


def host_prep(inputs, cfg):
    P, N, M, E, C, FP, FN, NC = (cfg[k] for k in
                                 ("P", "N", "M", "E", "C", "FP", "FN", "NC"))
    M_LOC = M // NC
    N_LOC = N // NC                      # 12500 planar nodes per core per plane
    NB = _ceil(M_LOC, 128)               # up seg blocks per core
    NBLK = _ceil(N_LOC, 128)             # down 128-src blocks (98)
    NP = NBLK * 128                      # padded nodes (12544)
    NG = _ceil(N_LOC, SG)                # down src groups (25)

    x = np.ascontiguousarray(np.asarray(inputs["x"], np.float32)
                             .reshape(P, N, C * FP))
    esrc = np.asarray(inputs["edge_src"])
    edst = np.asarray(inputs["edge_dst"])

    # per-core feature-major x slices, padded to NP: [P, C*FP, NP]
    xloc = np.zeros((NC, P, C * FP, NP), np.float32)
    for k in range(NC):
        xloc[k, :, :, :N_LOC] = (x[:, k * N_LOC:(k + 1) * N_LOC, :]
                                 .transpose(0, 2, 1))
    x_flat = x.reshape(P * N, C * FP)

    # ---------------- UP phase indices ----------------
    per_kp = {}
    for p in range(P):
        order = np.argsort(edst[p], kind="stable")
        ds, ss = edst[p][order], esrc[p][order]
        bounds = np.searchsorted(ds, np.arange(NC + 1) * M_LOC)
        for k in range(NC):
            sl = slice(bounds[k], bounds[k + 1])
            dsl = (ds[sl] - k * M_LOC).astype(np.int64)
            blk = dsl >> 7
            cnt = np.bincount(blk, minlength=NB)
            per_kp[(k, p)] = (dsl, (ss[sl] + p * N).astype(np.int64), blk, cnt)
    # per-(p, b) tile bound = max over cores
    KUPB = np.zeros((P, NB), np.int64)
    for p in range(P):
        for k in range(NC):
            cnt = per_kp[(k, p)][3]
            KUPB[p] = np.maximum(KUPB[p], (cnt + 127) // 128)
    KUPB = np.maximum(KUPB, 1)
    up_off = np.concatenate(([0], np.cumsum(KUPB.sum(axis=1))))  # per plane
    NUPT = int(up_off[-1])               # total up tiles over all planes

    up_src = np.zeros((NC, 128, NUPT), np.int32)
    up_dr = np.full((NC, 128, NUPT), -1.0, np.float32)
    for p in range(P):
        col0 = np.concatenate(([0], np.cumsum(KUPB[p])))[:-1] + up_off[p]
        for k in range(NC):
            dsl, sglob, blk, cnt = per_kp[(k, p)]
            starts = np.concatenate(([0], np.cumsum(cnt)))[:-1]
            r = np.arange(len(dsl)) - np.repeat(starts, cnt)
            col = col0[blk] + (r >> 7)
            lane = r & 127
            up_src[k, lane, col] = sglob
            up_dr[k, lane, col] = dsl - (blk << 7)

    # ---------------- DOWN phase indices ----------------
    # per (k, p): edges sorted by local src; groups of SG srcs; tiles of 128.
    down = {}
    cntg = np.zeros((NC, P, NG), np.int64)
    for p in range(P):
        owner = esrc[p] // N_LOC
        for k in range(NC):
            m = owner == k
            s_loc = (esrc[p][m] - k * N_LOC).astype(np.int64)
            d_glob = edst[p][m].astype(np.int64)
            o = np.argsort(s_loc, kind="stable")
            s_loc, d_glob = s_loc[o], d_glob[o]
            g = s_loc // SG
            cnt = np.bincount(g, minlength=NG)
            down[(k, p)] = (s_loc, d_glob, cnt)
            cntg[k, p] = cnt
    TPG = (cntg.max(axis=0) + 127) // 128        # [P, NG] tiles per group
    TPG = np.maximum(TPG, 1)
    dn_off = np.concatenate(([0], np.cumsum(TPG.sum(axis=1))))   # per plane
    NDNT = int(dn_off[-1])               # total down tiles over all planes

    dn_idx = np.zeros((NC, 128, NDNT), np.int32)
    dn_srel = np.full((NC, 128, NDNT), -1.0, np.float32)
    for p in range(P):
        col0 = np.concatenate(([0], np.cumsum(TPG[p])))[:-1] + dn_off[p]
        for k in range(NC):
            s_loc, d_glob, cnt = down[(k, p)]
            starts = np.concatenate(([0], np.cumsum(cnt)))[:-1]
            g = s_loc // SG
            r = np.arange(len(s_loc)) - np.repeat(starts, cnt)
            col = col0[g] + (r >> 7)
            lane = r & 127
            dn_idx[k, lane, col] = d_glob
            dn_srel[k, lane, col] = s_loc - g * SG

    # invdeg node-major [P, 128, NBLK]: (p, lane, b) = 1/max(deg[b*128+lane],1)
    invdeg = np.zeros((NC, P, 128, NBLK), np.float32)  # cast to bf16 below
    for p in range(P):
        for k in range(NC):
            s_loc = down[(k, p)][0]
            deg = np.bincount(s_loc, minlength=NP).astype(np.float32)
            invdeg[k, p] = (1.0 / np.maximum(deg, 1.0)).reshape(NBLK, 128).T

    # ---------------- weights ----------------
    g = lambda n: np.asarray(inputs[n], np.float32)
    Wn1, Wn2, We, Wd1, Wd2 = g("Wn1"), g("Wn2"), g("We"), g("Wd1"), g("Wd2")
    bn1, bn2, be, bd1, bd2 = g("bn1"), g("bn2"), g("be"), g("bd1"), g("bd2")

    wn1t = np.stack([Wn1.transpose(2, 0, 1)[p * FP:(p + 1) * FP]
                     .reshape(FP, C * FN) for p in range(P)]).copy()
    wn2t = Wn2.transpose(2, 0, 1).reshape(FN, C * FN).copy()
    # b-term weights: block-diagonal for classes 0..3 (K = 4*FN) and an
    # augmented [FN+1] block for class 4 whose ones-row adds be for all cols.
    went = We[:, :, 0, FP:]                                   # [P, C, FN]
    wentA = np.zeros((4 * FN, C * P), np.float32)
    for c in range(4):
        wentA[c * FN:(c + 1) * FN, c * P:(c + 1) * P] = went[:, c, :].T
    wentB = np.zeros((FN + 1, C * P), np.float32)
    wentB[:FN, 4 * P:] = went[:, 4, :].T
    wentB[FN, :] = be[:, :, 0].T.reshape(-1)
    bn1c = bn1.reshape(C, FN, 1).copy()
    bn2c = bn2.reshape(C, FN, 1).copy()
    # stage A block-diag weight [C*FP, C] (a = x . We_x per class)
    wblk = np.zeros((P, C * FP, C), np.float32)
    for p in range(P):
        for c in range(C):
            wblk[p, c * FP:(c + 1) * FP, c] = We[p, c, 0, :FP]
    # stage C weights, split x-part / n-part
    wd1x = Wd1[:, :, :, :FP].transpose(0, 1, 3, 2).copy()     # [P, C, FP, FP]
    wd1n = Wd1[:, :, :, FP:].transpose(0, 1, 3, 2)            # [P, C, FN, FP]
    wd1nA = np.ascontiguousarray(wd1n[:, :3].reshape(P, 3 * FN, FP)
                                 .astype(np.dtype(ml_dtypes.bfloat16)))
    wd1nB = np.ascontiguousarray(wd1n[:, 3:5].reshape(P, 2 * FN, FP)
                                 .astype(np.dtype(ml_dtypes.bfloat16)))
    wd2t = Wd2.transpose(0, 1, 3, 2).copy()                   # [P, C, FP, FP]
    bd1c = bd1.reshape(P, C, FP, 1).copy()
    bd2c = bd2.reshape(P, C, FP, 1).copy()
    iota = np.tile(np.arange(128, dtype=np.float32), (128, 1)).copy()
    iota512 = np.tile(np.arange(SG, dtype=np.float32), (128, 1)).copy()
    ident = np.eye(128, dtype=np.float32)

    meta = dict(cfg=cfg, M_LOC=M_LOC, N_LOC=N_LOC, NB=NB, NBLK=NBLK, NP=NP,
                NG=NG, KUPB=KUPB, up_off=up_off, NUPT=NUPT,
                TPG=TPG, dn_off=dn_off, NDNT=NDNT)

    shared = dict(x=x_flat, wn1t=wn1t, wn2t=wn2t, wentA=wentA, wentB=wentB,
                  bn1c=bn1c, bn2c=bn2c, wblk=wblk, wd1x=wd1x, wd1nA=wd1nA,
                  wd1nB=wd1nB, wd2t=wd2t, bd1c=bd1c, bd2c=bd2c, iota=iota,
                  iota512=iota512, ident=ident)
    in_maps = []
    for k in range(NC):
        m = dict(shared)
        ivh = invdeg[k].astype(np.dtype(ml_dtypes.bfloat16))
        ivl = (invdeg[k] - ivh.astype(np.float32)).astype(
            np.dtype(ml_dtypes.bfloat16))
        m.update(xloc=xloc[k], up_src=up_src[k], up_dr=up_dr[k],
                 dn_idx=dn_idx[k], dn_srel=dn_srel[k],
                 invdeg=ivh, invdeglo=ivl)
        in_maps.append(m)
    return in_maps, meta


def build_kernel(meta):
    cfg = meta["cfg"]
    P, N, M, E, C, FP, FN, NC = (cfg[k] for k in
                                 ("P", "N", "M", "E", "C", "FP", "FN", "NC"))
    M_LOC, NB = meta["M_LOC"], meta["NB"]
    NBLK, NP, NG = meta["NBLK"], meta["NP"], meta["NG"]
    KUPB, up_off, NUPT = meta["KUPB"], meta["up_off"], meta["NUPT"]
    TPG, dn_off, NDNT = meta["TPG"], meta["dn_off"], meta["NDNT"]
    CF = C * FP
    CN = C * FN
    assert C == 5
    NCH = _ceil(NP, CHW)                 # stage A/C chunks (NP/512 -> 24.5)

    nc = bacc.Bacc("TRN2", num_devices=NC)

    def param(name, shape, dt=F32, out=False):
        return nc.declare_dram_parameter(name, list(shape), dt, isOutput=out)

    x_d = param("x", [P * N, CF])
    xloc_d = param("xloc", [P, CF, NP])
    up_src_d = param("up_src", [128, NUPT], I32)
    up_dr_d = param("up_dr", [128, NUPT])
    dn_idx_d = param("dn_idx", [128, NDNT], I32)
    dn_srel_d = param("dn_srel", [128, NDNT])
    invdeg_d = param("invdeg", [P, 128, NBLK], BF16)
    invdeglo_d = param("invdeglo", [P, 128, NBLK], BF16)
    wn1t_d = param("wn1t", [P, FP, CN])
    wn2t_d = param("wn2t", [FN, CN])
    wentA_d = param("wentA", [4 * FN, C * P])
    wentB_d = param("wentB", [FN + 1, C * P])
    bn1c_d = param("bn1c", [C, FN, 1])
    bn2c_d = param("bn2c", [C, FN, 1])
    wblk_d = param("wblk", [P, CF, C])
    wd1x_d = param("wd1x", [P, C, FP, FP])
    wd1nA_d = param("wd1nA", [P, 3 * FN, FP], BF16)
    wd1nB_d = param("wd1nB", [P, 2 * FN, FP], BF16)
    wd2t_d = param("wd2t", [P, C, FP, FP])
    bd1c_d = param("bd1c", [P, C, FP, 1])
    bd2c_d = param("bd2c", [P, C, FP, 1])
    iota_d = param("iota", [128, 128])
    iota512_d = param("iota512", [128, SG])
    ident_d = param("ident", [128, 128])
    out_d = param("outT", [P, C, FP, NP], out=True)
    DBG = globals().get("DEBUG_OUT", False)
    if DBG:
        nloc_dbg = param("nloc_dbg", [M_LOC, NROW], out=True)
        asb_dbg = param("asb_dbg", [128, NBLK * 6], out=True)
        agg_dbg = param("agg_dbg", [3 * FN, NP], out=True)
        la_dbg = param("la_dbg", [128, 64, 6], out=True)

    n_loc = nc.dram_tensor("n_loc", [M_LOC, NROW], F32)
    n_full = nc.dram_tensor("n_full", [NC * M_LOC, NROW], F32,
                            addr_space="Shared")

    with tile.TileContext(nc) as tc:
        with tc.tile_pool(name="const", bufs=1) as cp:
            iota_t = cp.tile([128, 128], F32R)
            nc.sync.dma_start(out=iota_t[:], in_=iota_d[:].bitcast(F32R))
            iota512_t = cp.tile([128, SG], F32)
            nc.sync.dma_start(out=iota512_t[:], in_=iota512_d[:])
            ident_t = cp.tile([128, 128], F32)
            nc.sync.dma_start(out=ident_t[:], in_=ident_d[:])
            ident_bf = cp.tile([128, 128], BF16)
            nc.vector.tensor_copy(out=ident_bf[:], in_=ident_t[:])
            wn1t_t = [cp.tile([FP, CN], F32R, name=f"wn1t{p}")
                      for p in range(P)]
            wn2t_t = cp.tile([FN, CN], F32R)
            wentA_t = cp.tile([4 * FN, C * P], F32R)
            wentB_t = cp.tile([FN + 1, C * P], F32R)
            nc.sync.dma_start(out=wn2t_t[:], in_=wn2t_d[:].bitcast(F32R))
            nc.sync.dma_start(out=wentA_t[:], in_=wentA_d[:].bitcast(F32R))
            nc.sync.dma_start(out=wentB_t[:], in_=wentB_d[:].bitcast(F32R))
            bn1c_t = [cp.tile([FN, 1], F32, name=f"bn1c{c}") for c in range(C)]
            bn2c_t = [cp.tile([FN, 1], F32, name=f"bn2c{c}") for c in range(C)]
            wblk_t = [[cp.tile([min(128, CF - ti * 128), C], F32R,
                               name=f"wblk{p}_{ti}") for ti in range(3)]
                      for p in range(P)]
            wd1x_t = [[cp.tile([FP, FP], F32R, name=f"wd1x{p}_{c}")
                       for c in range(C)] for p in range(P)]
            wd1nA_t = [cp.tile([3 * FN, FP], BF16, name=f"wd1nA{p}")
                       for p in range(P)]
            wd1nB_t = [cp.tile([2 * FN, FP], BF16, name=f"wd1nB{p}")
                       for p in range(P)]
            wd2t_t = [[cp.tile([FP, FP], F32R, name=f"wd2t{p}_{c}")
                       for c in range(C)] for p in range(P)]
            bd1c_t = [[cp.tile([FP, 1], F32, name=f"bd1c{p}_{c}")
                       for c in range(C)] for p in range(P)]
            bd2c_t = [[cp.tile([FP, 1], F32, name=f"bd2c{p}_{c}")
                       for c in range(C)] for p in range(P)]
            for p in range(P):
                nc.sync.dma_start(out=wn1t_t[p][:], in_=wn1t_d[p].bitcast(F32R))
                nc.sync.dma_start(out=wd1nA_t[p][:], in_=wd1nA_d[p])
                nc.sync.dma_start(out=wd1nB_t[p][:], in_=wd1nB_d[p])
                for ti in range(3):
                    w = min(128, CF - ti * 128)
                    nc.sync.dma_start(
                        out=wblk_t[p][ti][:],
                        in_=wblk_d[p, ti * 128:ti * 128 + w, :].bitcast(F32R))
                for c in range(C):
                    nc.sync.dma_start(out=wd1x_t[p][c][:],
                                      in_=wd1x_d[p, c].bitcast(F32R))
                    nc.sync.dma_start(out=wd2t_t[p][c][:],
                                      in_=wd2t_d[p, c].bitcast(F32R))
                    nc.sync.dma_start(out=bd1c_t[p][c][:], in_=bd1c_d[p, c])
                    nc.sync.dma_start(out=bd2c_t[p][c][:], in_=bd2c_d[p, c])
            for c in range(C):
                nc.sync.dma_start(out=bn1c_t[c][:], in_=bn1c_d[c])
                nc.sync.dma_start(out=bn2c_t[c][:], in_=bn2c_d[c])
            upsrc_t = cp.tile([128, NUPT], I32)
            updr_t = cp.tile([128, NUPT], F32)
            nc.scalar.dma_start(out=upsrc_t[:], in_=up_src_d[:])
            nc.scalar.dma_start(out=updr_t[:], in_=up_dr_d[:])
            dnidx_t = cp.tile([128, NDNT], I32)
            dnsrel_t = cp.tile([128, NDNT], F32)
            nc.scalar.dma_start(out=dnidx_t[:], in_=dn_idx_d[:])
            nc.scalar.dma_start(out=dnsrel_t[:], in_=dn_srel_d[:])
            ones_f = cp.tile([1, GRP * 128], F32)
            nc.vector.memset(ones_f[:], 1.0)
            ones_r = cp.tile([1, GRP * 128], F32R)
            nc.vector.tensor_copy(out=ones_r[:], in_=ones_f[:])

            # ======================= UP PHASE =======================
            n_loc_ap = n_loc.ap()
            with tc.tile_pool(name="up_sb", bufs=3) as up, \
                 tc.tile_pool(name="up_sb1", bufs=2) as up1, \
                 tc.tile_pool(name="up_ps", bufs=2, space="PSUM") as upp, \
                 tc.tile_pool(name="up_ps1", bufs=1, space="PSUM") as upp1, \
                 tc.tile_pool(name="mlp_ps", bufs=1, space="PSUM") as mpp:
                for g0 in range(0, NB, GRP):
                    gb = list(range(g0, min(g0 + GRP, NB)))
                    GW = len(gb) * 128
                    # per-plane per-class feature-major up tiles [64, GRP*128]
                    upX = [[up1.tile([FP, GRP * 128], F32R,
                                     name=f"upX{p}_{c}", tag=f"upX{p}_{c}")
                            for c in range(C)] for p in range(P)]
                    for p in range(P):
                        for bi, b in enumerate(gb):
                            pu = upp.tile([128, CF], F32, tag="pu",
                                          space="PSUM")
                            col_b = up_off[p] + int(KUPB[p][:b].sum())
                            KUP = int(KUPB[p][b])
                            for kk in range(KUP):
                                col = col_b + kk
                                G = up.tile([128, CF], F32R, tag="G")
                                nc.gpsimd.indirect_dma_start(
                                    out=G[:], out_offset=None,
                                    in_=x_d[:].bitcast(F32R),
                                    in_offset=bass.IndirectOffsetOnAxis(
                                        ap=upsrc_t[:, col:col + 1], axis=0))
                                O = up.tile([128, 128], F32R, tag="O")
                                nc.vector.tensor_tensor(
                                    out=O[:],
                                    in0=updr_t[:, col:col + 1]
                                        .bitcast(F32R).to_broadcast([128, 128]),
                                    in1=iota_t[:],
                                    op=ALU.is_equal)
                                nc.tensor.matmul(out=pu[:], lhsT=O[:],
                                                 rhs=G[:], start=(kk == 0),
                                                 stop=(kk == KUP - 1))
                            stg = up.tile([128, CF], F32, tag="stg")
                            nc.scalar.copy(out=stg[:], in_=pu[:])
                            csl = slice(bi * 128, (bi + 1) * 128)
                            for ti in range(3):
                                w = min(128, CF - ti * 128)
                                pt = upp1.tile([128, 128], F32, tag="ptr",
                                               space="PSUM")
                                nc.tensor.transpose(
                                    out=pt[:w, :],
                                    in_=stg[:, ti * 128:ti * 128 + w],
                                    identity=ident_t[:])
                                nc.vector.tensor_copy(
                                    out=upX[p][2 * ti][:, csl],
                                    in_=pt[0:FP, :])
                                if 2 * ti + 1 < C:
                                    nc.vector.tensor_copy(
                                        out=upX[p][2 * ti + 1][:, csl],
                                        in_=pt[FP:2 * FP, :])
                    # ---- nexus MLP over this group ----
                    n1c = [up.tile([FN, GRP * 128], F32R, name=f"n1c{c}",
                                   tag=f"n1c{c}") for c in range(C)]
                    for c in range(C):
                        pn1 = mpp.tile([FN, GRP * 128], F32, tag="pn1",
                                       space="PSUM", bufs=2)
                        for p in range(P):
                            nc.tensor.matmul(
                                out=pn1[:, :GW],
                                lhsT=wn1t_t[p][:, c * FN:(c + 1) * FN],
                                rhs=upX[p][c][:, :GW],
                                start=(p == 0), stop=(p == P - 1))
                        nc.scalar.activation(n1c[c][:, :GW], pn1[:, :GW],
                                             TANH, bias=bn1c_t[c][:])
                    n2s = up.tile([4 * FN, GRP * 128], F32R, tag="n2s")
                    nbt = up.tile([FN + 1, GRP * 128], F32R, tag="nbt")
                    nc.vector.tensor_copy(out=nbt[FN:FN + 1, :],
                                          in_=ones_r[:])
                    for c in range(C):
                        pn2 = mpp.tile([FN, GRP * 128], F32, tag="pn2",
                                       space="PSUM", bufs=2)
                        nc.tensor.matmul(
                            out=pn2[:, :GW],
                            lhsT=wn2t_t[:, c * FN:(c + 1) * FN],
                            rhs=n1c[c][:, :GW], start=True, stop=True)
                        dst = (n2s[c * FN:(c + 1) * FN, :GW] if c < 4
                               else nbt[0:FN, :GW])
                        nc.scalar.activation(dst, pn2[:, :GW],
                                             TANH, bias=bn2c_t[c][:])
                    pbv = mpp.tile([C * P, GRP * 128], F32, tag="misc",
                                   space="PSUM", bufs=1)
                    nc.tensor.matmul(out=pbv[:, :GW], lhsT=wentA_t[:],
                                     rhs=n2s[:, :GW], start=True, stop=False)
                    nc.tensor.matmul(out=pbv[:, :GW], lhsT=wentB_t[:],
                                     rhs=nbt[:, :GW], start=False, stop=True)
                    bt = up.tile([C * P, GRP * 128], F32, tag="bt")
                    nc.vector.tensor_copy(out=bt[:, :GW], in_=pbv[:, :GW])
                    # assemble + store n rows per block
                    for bi, b in enumerate(gb):
                        rows = min(128, M_LOC - b * 128)
                        sl = slice(bi * 128, bi * 128 + 128)
                        tp = mpp.tile([128, 4 * FN + FN + C * P], F32,
                                      tag="misc", space="PSUM", bufs=1)
                        nc.tensor.transpose(
                            out=tp[:, 0:4 * FN],
                            in_=n2s[:, sl].bitcast(F32),
                            identity=ident_t[:])
                        nc.tensor.transpose(
                            out=tp[:, 4 * FN:CN],
                            in_=nbt[0:FN, sl].bitcast(F32),
                            identity=ident_t[:FN, :FN])
                        nc.tensor.transpose(
                            out=tp[:, CN:CN + C * P],
                            in_=bt[:, sl],
                            identity=ident_t[:C * P, :C * P])
                        nrow = up.tile([128, NROW], F32, tag="nrow")
                        nc.vector.tensor_copy(out=nrow[:, 0:CN + C * P],
                                              in_=tp[:])
                        nc.vector.memset(nrow[:, CN + C * P:], 0.0)
                        nc.sync.dma_start(
                            out=n_loc_ap[b * 128:b * 128 + rows, :],
                            in_=nrow[:rows, :])

            if DBG:
                with tc.tile_pool(name="dbg", bufs=1) as dbp:
                    for b0 in range(0, M_LOC, 128):
                        rows = min(128, M_LOC - b0)
                        dt_ = dbp.tile([128, NROW], F32, tag="dt")
                        nc.sync.dma_start(out=dt_[:rows], in_=n_loc.ap()[b0:b0 + rows])
                        nc.sync.dma_start(out=nloc_dbg[b0:b0 + rows], in_=dt_[:rows])

            # ================= AllGather n =================
            nc.gpsimd.collective_compute(
                "AllGather", ALU.bypass,
                replica_groups=[list(range(NC))],
                ins=[n_loc.ap().opt()], outs=[n_full.ap().opt()])

            # ============ DOWN: per plane, A -> B -> C ============
            with tc.tile_pool(name="sa_sb", bufs=3) as sa, \
                 tc.tile_pool(name="asb", bufs=1) as asbp, \
                 tc.tile_pool(name="ft", bufs=1) as ftp, \
                 tc.tile_pool(name="sb_g", bufs=2) as sbg, \
                 tc.tile_pool(name="sb_t", bufs=4) as sbt, \
                 tc.tile_pool(name="sc_sb", bufs=3) as scb, \
                 tc.tile_pool(name="sa_ps", bufs=1, space="PSUM") as sap, \
                 tc.tile_pool(name="sb_ps", bufs=1, space="PSUM") as sbp, \
                 tc.tile_pool(name="sc_ps", bufs=2, space="PSUM") as scp:
                for p in range(P):
                    # -------- stage A: a-table [128, NBLK, 6] --------
                    a_sb = asbp.tile([128, NBLK, 6], BF16, tag="a_sb")
                    a_lo = asbp.tile([128, NBLK, 6], BF16, tag="a_lo")
                    nc.sync.dma_start(
                        out=a_sb[:, :, 5:6],
                        in_=invdeg_d[p].rearrange("q (b o) -> q b o", o=1))
                    nc.sync.dma_start(
                        out=a_lo[:, :, 5:6],
                        in_=invdeglo_d[p].rearrange("q (b o) -> q b o", o=1))
                    for ch in range(NCH):
                        cw = min(CHW, NP - ch * CHW)
                        nch = cw // 128
                        pa = sap.tile([C, CHW], F32, tag="pa", space="PSUM")
                        for ti in range(3):
                            w = min(128, CF - ti * 128)
                            xa = sa.tile([128, CHW], F32R, tag="xa")
                            nc.sync.dma_start(
                                out=xa[:w, :cw],
                                in_=xloc_d[p, ti * 128:ti * 128 + w,
                                           ch * CHW:ch * CHW + cw]
                                    .bitcast(F32R))
                            nc.tensor.matmul(
                                out=pa[:, :cw],
                                lhsT=wblk_t[p][ti][:],
                                rhs=xa[:w, :cw],
                                start=(ti == 0), stop=(ti == 2))
                        af = sa.tile([C, CHW], F32, tag="af")
                        nc.vector.tensor_copy(out=af[:, :cw], in_=pa[:, :cw])
                        for j in range(nch):
                            pt = sap.tile([128, C], F32, tag="pat",
                                          space="PSUM")
                            nc.tensor.transpose(
                                out=pt[:, :],
                                in_=af[:, j * 128:(j + 1) * 128],
                                identity=ident_t[:C, :C])
                            nc.vector.tensor_copy(
                                out=a_sb[:, ch * 4 + j, 0:5], in_=pt[:])
                            ahf = sa.tile([128, C], F32, tag="ahf")
                            nc.vector.tensor_copy(
                                out=ahf[:], in_=a_sb[:, ch * 4 + j, 0:5])
                            nc.vector.tensor_tensor(
                                out=a_lo[:, ch * 4 + j, 0:5],
                                in0=pt[:], in1=ahf[:], op=ALU.subtract)

                    if DBG and p == 0:
                        af32 = scb.tile([128, NBLK * 6], F32, tag="afx")
                        nc.vector.tensor_copy(
                            out=af32[:],
                            in_=a_sb[:].rearrange("q b s -> q (b s)"))
                        nc.sync.dma_start(out=asb_dbg[:], in_=af32[:])
                    # -------- stage B: gather, weights, aggregate --------
                    aggA = ftp.tile([3 * FN, NP], BF16, tag="aggA")
                    aggB = ftp.tile([2 * FN, NP], BF16, tag="aggB")
                    for g in range(NG):
                        T = int(TPG[p][g])
                        col0 = dn_off[p] + int(TPG[p][:g].sum())
                        gn = sbg.tile([128, T, NROW], F32, tag="gn", bufs=2)
                        oh = sbg.tile([128, T, SG], BF16, tag="oh", bufs=2)
                        la = sbg.tile([128, T, 6], F32, tag="la", bufs=2)
                        for t in range(T):
                            col = col0 + t
                            nc.gpsimd.indirect_dma_start(
                                out=gn[:, t, :], out_offset=None,
                                in_=n_full.ap(),
                                in_offset=bass.IndirectOffsetOnAxis(
                                    ap=dnidx_t[:, col:col + 1], axis=0))
                            nc.vector.tensor_tensor(
                                out=oh[:, t, :],
                                in0=dnsrel_t[:, col:col + 1]
                                    .to_broadcast([128, SG]),
                                in1=iota512_t[:],
                                op=ALU.is_equal)
                            # transpose one-hot chunks, expand a to edges
                            ohT = sbt.tile([128, 4, 128], BF16, tag="ohT")
                            pla = sbp.tile([128, 6], F32, tag="pla",
                                           space="PSUM", bufs=1)
                            NCQ = min(4, NBLK - g * 4)
                            for cq in range(NCQ):
                                pT = sbp.tile([128, 128], BF16, tag="pT",
                                              space="PSUM", bufs=1)
                                nc.tensor.transpose(
                                    out=pT[:],
                                    in_=oh[:, t, cq * 128:(cq + 1) * 128],
                                    identity=ident_bf[:])
                                nc.vector.tensor_copy(
                                    out=ohT[:, cq, :], in_=pT[:])
                                nc.tensor.matmul(
                                    out=pla[:],
                                    lhsT=ohT[:, cq, :],
                                    rhs=a_sb[:, g * 4 + cq, :],
                                    start=(cq == 0), stop=False)
                                nc.tensor.matmul(
                                    out=pla[:],
                                    lhsT=ohT[:, cq, :],
                                    rhs=a_lo[:, g * 4 + cq, :],
                                    start=False, stop=(cq == NCQ - 1))
                            nc.vector.tensor_copy(out=la[:, t, :], in_=pla[:])
                            if DBG and p == 0 and g < 8 and t == 0:
                                nc.sync.dma_start(out=la_dbg[:, g, :],
                                                  in_=la[:, t, :])
                        # softmax weights over classes (logit = a + b)
                        lg = sbt.tile([128, T, C], F32, tag="lg")
                        nc.vector.tensor_tensor(
                            out=lg[:], in0=la[:, :, 0:C],
                            in1=gn[:, :, CN + p:CN + p + (C - 1) * P + 1:P],
                            op=ALU.add)
                        mx = sbt.tile([128, T], F32, tag="mx")
                        nc.vector.tensor_reduce(out=mx[:], in_=lg[:],
                                                axis=mybir.AxisListType.X,
                                                op=ALU.max)
                        nc.vector.tensor_tensor(
                            out=lg[:], in0=lg[:],
                            in1=mx[:].to_broadcast([128, T, C]),
                            op=ALU.subtract)
                        ex = sbt.tile([128, T, C], F32, tag="ex")
                        nc.scalar.activation(ex[:], lg[:], EXP)
                        sm = sbt.tile([128, T], F32, tag="sm")
                        nc.vector.tensor_reduce(out=sm[:], in_=ex[:],
                                                axis=mybir.AxisListType.X,
                                                op=ALU.add)
                        nc.vector.reciprocal(out=sm[:], in_=sm[:])
                        nc.vector.tensor_tensor(out=sm[:], in0=sm[:],
                                                in1=la[:, :, 5],
                                                op=ALU.mult)
                        nc.vector.tensor_tensor(
                            out=ex[:], in0=ex[:],
                            in1=sm[:].to_broadcast([128, T, C]),
                            op=ALU.mult)
                        msg = sbg.tile([128, T, CN], BF16, tag="msg", bufs=2)
                        nc.vector.tensor_tensor(
                            out=msg[:].rearrange(
                                "a b (c f) -> a b c f", f=FN),
                            in0=gn[:, :, 0:CN].rearrange(
                                "a b (c f) -> a b c f", f=FN),
                            in1=ex[:].to_broadcast([128, T, C, FN]),
                            op=ALU.mult)
                        # aggregate: feature-major one-hot matmuls
                        pF = sbp.tile([128, SG], F32, tag="pF",
                                      space="PSUM", bufs=1)
                        pF4 = sbp.tile([FN, SG], F32, tag="pF4",
                                       space="PSUM", bufs=1)
                        for t in range(T):
                            nc.tensor.matmul(
                                out=pF[:], lhsT=msg[:, t, 0:128],
                                rhs=oh[:, t, :],
                                start=(t == 0), stop=(t == T - 1))
                            nc.tensor.matmul(
                                out=pF4[:], lhsT=msg[:, t, 128:CN],
                                rhs=oh[:, t, :],
                                start=(t == 0), stop=(t == T - 1))
                        gw = min(SG, NP - g * SG)
                        nc.vector.tensor_copy(
                            out=aggA[:, g * SG:g * SG + gw],
                            in_=pF[0:3 * FN, :gw])
                        nc.vector.tensor_copy(
                            out=aggB[0:FN, g * SG:g * SG + gw],
                            in_=pF[3 * FN:4 * FN, :gw])
                        nc.vector.tensor_copy(
                            out=aggB[FN:2 * FN, g * SG:g * SG + gw],
                            in_=pF4[:, :gw])

                    if DBG and p == 0:
                        with tc.tile_pool(name="dbg2", bufs=1) as db2:
                            for q0 in range(0, NP, 2048):
                                qw = min(2048, NP - q0)
                                ag32 = db2.tile([3 * FN, 2048], F32, tag="agx")
                                nc.vector.tensor_copy(out=ag32[:, :qw],
                                                      in_=aggA[:, q0:q0 + qw])
                                nc.sync.dma_start(out=agg_dbg[:, q0:q0 + qw],
                                                  in_=ag32[:, :qw])
                    # -------- stage C: down MLP --------
                    for ch in range(NCH):
                        cw = min(CHW, NP - ch * CHW)
                        csl = slice(ch * CHW, ch * CHW + cw)
                        for c in range(C):
                            xc = scb.tile([FP, CHW], F32R, tag="xc")
                            nc.sync.dma_start(
                                out=xc[:, :cw],
                                in_=xloc_d[p, c * FP:(c + 1) * FP, csl]
                                    .bitcast(F32R))
                            hps = scp.tile([FP, CHW], F32, tag="hps",
                                           space="PSUM", bufs=1)
                            nc.tensor.matmul(
                                out=hps[:, :cw], lhsT=wd1x_t[p][c][:],
                                rhs=xc[:, :cw], start=True, stop=False)
                            if c < 3:
                                rhsn = aggA[c * FN:(c + 1) * FN, csl]
                                lhsn = wd1nA_t[p][c * FN:(c + 1) * FN, :]
                            else:
                                rhsn = aggB[(c - 3) * FN:(c - 2) * FN, csl]
                                lhsn = wd1nB_t[p][(c - 3) * FN:(c - 2) * FN, :]
                            nc.tensor.matmul(
                                out=hps[:, :cw], lhsT=lhsn,
                                rhs=rhsn, start=False, stop=True)
                            ht = scb.tile([FP, CHW], F32R, tag="ht")
                            nc.scalar.activation(ht[:, :cw], hps[:, :cw],
                                                 TANH, bias=bd1c_t[p][c][:])
                            ops_ = scp.tile([FP, CHW], F32, tag="ops",
                                            space="PSUM", bufs=1)
                            nc.tensor.matmul(
                                out=ops_[:, :cw], lhsT=wd2t_t[p][c][:],
                                rhs=ht[:, :cw], start=True, stop=True)
                            ot = scb.tile([FP, CHW], F32, tag="ot")
                            nc.scalar.activation(ot[:, :cw], ops_[:, :cw],
                                                 TANH, bias=bd2c_t[p][c][:])
                            nc.sync.dma_start(
                                out=out_d[p, c, :, csl],
                                in_=ot[:, :cw])

    nc.compile()
    return nc


_CACHE = {}


def _get_compiled(inputs, cfg):
    in_maps, meta = host_prep(inputs, cfg)
    key = (meta["NUPT"], meta["NDNT"], tuple(sorted(cfg.items())))
    if key not in _CACHE:
        _CACHE[key] = build_kernel(meta)
    return _CACHE[key], in_maps, meta


def assemble_output(results, meta):
    cfg = meta["cfg"]
    P, N, C, FP, NC = (cfg[k] for k in ("P", "N", "C", "FP", "NC"))
    N_LOC = meta["N_LOC"]
    # results[k]["outT"]: [P, C, FP, NP]
    arr = np.stack([np.asarray(results[k]["outT"])[:, :, :, :N_LOC]
                    for k in range(NC)])
    # [NC, P, C, FP, NL] -> [P, NC, NL, C, FP]
    out = arr.transpose(1, 0, 4, 2, 3).reshape(P, N, C, FP)
    return np.ascontiguousarray(out)


def kernel(**inputs):
    from concourse.bass_utils import run_bass_kernel_spmd
    cfg = CFG_FULL
    nc, in_maps, meta = _get_compiled(inputs, cfg)
    res = run_bass_kernel_spmd(nc, in_maps, list(range(cfg["NC"])))
    return assemble_output(res.results, meta)
